# revision 35
# baseline (speedup 1.0000x reference)
"""Trainium2 Bass kernel for nn_AutoCorrelation_spa_tem.

Shards batch B=32 across 8 NeuronCores (4 batches/core, pure data parallel).

Algorithm (collapsed form of the reference):
  G_b   = keys[b](L,HE) @ queries[b](L,HE)^T            (192x192)
  D_raw[b,tau] = sum_s G_b[s,(s+tau)%L]                 (diag sums via shear)
  gsum  = AllGather_b(sum_b D_raw) + local sum -> top-5 mask via max8
  c_b   = mask * softmax(D_raw[b]/HE over selected)
  W_b   = keys[b].reshape(HE,L)^T @ values_proper(HE,L) (192x192)
  M_b   = sum_d c_b[d] * Shift_d(W_b)   [2D circular diagonal shift]
        = unshear(HankelC^T @ shear(W_b))   (all positive-stride DMAs)
  out[b] = (Qtilde_b @ M_b)^T  computed as Mrev^T @ qr  (qr host-row-reversed)

Perf structure vs the naive version:
  - all matmul operands in bf16 (tolerance is 2e-2; end-to-end err ~5e-3)
  - DMAs batched across the 4 local batches with 3-level access patterns
    (~30 dma_starts total; descriptor-generation on the issuing engine is
    ~0.6-1us per dma_start and was the old bottleneck)
  - collective input computed from Gsum = sum_b G_b (one small fp32 shear
    bounce) so the AllGather triggers as early as possible; everything else
    (per-batch D, W phase) runs during collective flight
  - dma_starts spread across sync(SP-HWDGE) / scalar(Act-HWDGE) / gpsimd
"""

import numpy as np

B, L, H, E = 32, 192, 8, 64
HE = H * E
N_CORES = 8
PER = B // N_CORES
L2 = 2 * L
BSTR = L * 576            # per-batch stride in shear scratch arrays

USE_ALLGATHER = True
USE_RDMA = False          # butterfly all-reduce via remote_dma: hits an opaque
                          # NRT INTERNAL error on this runtime; keep off
N_WARM = 8                # PE pstate warm-up matmuls

_compiled = {}


def _build():
    import concourse.bacc as bacc
    import concourse.mybir as mybir
    from concourse.bass_types import AP
    from concourse.tile import TileContext, add_dep_helper

    f32 = mybir.dt.float32
    bf = mybir.dt.bfloat16
    Exp = mybir.ActivationFunctionType.Exp
    CopyF = mybir.ActivationFunctionType.Copy
    Alu = mybir.AluOpType
    Ax = mybir.AxisListType

    nc = bacc.Bacc("TRN2", target_bir_lowering=False, debug=False,
                   num_devices=N_CORES, num_swdge_queues=2)

    # ---- dram I/O (host-packed layouts, see kernel()) ----
    kt_d = nc.dram_tensor("kt", [128, PER * 4 * L], bf, kind="ExternalInput")
    qt_d = nc.dram_tensor("qt", [128, PER * 4 * L], bf, kind="ExternalInput")
    kf_d = nc.dram_tensor("kf", [128, PER * 4 * L], bf, kind="ExternalInput")
    vt_d = nc.dram_tensor("vt", [128, PER * 4 * L], bf, kind="ExternalInput")
    qr0_d = nc.dram_tensor("qr0", [128, PER * HE], bf, kind="ExternalInput")
    qr1_d = nc.dram_tensor("qr1", [64, PER * HE], bf, kind="ExternalInput")
    out_d = nc.dram_tensor("out", [PER, L, HE], bf, kind="ExternalOutput")

    # ---- dram scratch ----
    gs3 = nc.dram_tensor("gs3", [L * 576 + 640], f32)          # Gsum shear bounce
    g3a = nc.dram_tensor("g3a", [PER * BSTR + 640], f32)       # per-batch G bounce
    w3 = nc.dram_tensor("w3", [PER * BSTR + 640], bf)          # W shear bounce
    m3a = nc.dram_tensor("m3a", [PER * BSTR + 640], bf)        # M unshear bounce
    c3a = nc.dram_tensor("c3a", [PER * 576 + 640], bf)         # tripled c for circulant
    arin = nc.dram_tensor("arin", [1, L], f32)
    if USE_ALLGATHER:
        arout = nc.dram_tensor("arout", [N_CORES, L], f32, addr_space="Shared")
    else:
        arout = nc.dram_tensor("arout", [1, L], f32, addr_space="Shared")

    PCH = [(0, 128), (128, 64)]
    rdma_posthoc = []

    with TileContext(nc) as tc:
        with tc.tile_pool(name="sb", bufs=1) as sb, \
             tc.tile_pool(name="ps", bufs=1, space="PSUM") as ps:

            # ================= constants (off critical path) =================
            warm_t = sb.tile([128, 256], bf, tag="warm_t")
            nc.vector.memset(warm_t[:, :], 0.125)
            ones_t = sb.tile([128, 1], f32, tag="ones")
            nc.vector.memset(ones_t[:, :], 1.0)
            oh_t = sb.tile([128, 16], f32, tag="oh")   # one-hot blocks: col 4b+b is 1
            nc.vector.memset(oh_t[:, :], 0.0)
            for b in range(PER):
                nc.vector.memset(oh_t[:, 4 * b + b: 4 * b + b + 1], 1.0)
            ones14 = sb.tile([1, 4], f32, tag="ones14")
            nc.vector.memset(ones14[:, :], 1.0)
            if USE_RDMA:
                ones_t2 = sb.tile([128, 128], f32, tag="ones_t2")
                nc.vector.memset(ones_t2[:, :], 1.0)
                # rs[k] is bumped only by the round-k partner; cleared right
                # after consumption (NOT at start: a partner leading by the
                # launch skew may bump before our preamble runs). Inboxes are
                # deliberately never written locally for the same reason.
                rs = [nc.alloc_semaphore(f"bfly_r{k}") for k in range(3)]
                ls_sem = nc.alloc_semaphore("bfly_l")
                inbox = []
                for k in range(3):
                    ib = sb.tile([128, L], f32, tag=f"inbox{k}", name=f"inbox{k}")
                    inbox.append(ib)

            # ================= input loads (G-phase operands only) =================
            HL = PER * 4 * L // 2
            kt_t = sb.tile([128, PER * 4 * L], bf, tag="kt_t")
            qt_t = sb.tile([128, PER * 4 * L], bf, tag="qt_t")
            kf_t = sb.tile([128, PER * 4 * L], bf, tag="kf_t")
            vt_t = sb.tile([128, PER * 4 * L], bf, tag="vt_t")
            qr0_t = sb.tile([128, PER * HE], bf, tag="qr0_t")
            qr1_t = sb.tile([64, PER * HE], bf, tag="qr1_t")
            nc.sync.dma_start(out=kt_t[:, 0:HL], in_=kt_d[:, 0:HL])
            nc.gpsimd.dma_start(out=qt_t[:, 0:HL], in_=qt_d[:, 0:HL])
            nc.sync.dma_start(out=kt_t[:, HL:], in_=kt_d[:, HL:])
            nc.gpsimd.dma_start(out=qt_t[:, HL:], in_=qt_d[:, HL:])

            # ================= PE warm-up =================
            for w in range(N_WARM):
                wp = ps.tile([128, 256], f32, tag="warm_p", bufs=1)
                nc.tensor.matmul(wp[:, :], warm_t[:, 0:128], warm_t[:, :],
                                 start=True, stop=True)

            # ================= G matmuls + dup copies + Gsum =================
            gcp = {}   # (chunk) -> tile (mn, PER*2L) f32, per-batch rows duplicated
            for ci, (m0, mn) in enumerate(PCH):
                gcp[m0] = sb.tile([mn, PER * L2], f32, tag=f"gcp{m0}", name=f"gcp{m0}")
            gsm = {}   # Gsum per chunk (mn, L) f32
            for b in range(PER):
                for ci, (m0, mn) in enumerate(PCH):
                    gp = ps.tile([mn, L], f32, tag="mm", bufs=4)
                    for i in range(4):
                        nc.tensor.matmul(
                            gp[:, :],
                            kt_t[:, (b * 4 + i) * L + m0:(b * 4 + i) * L + m0 + mn],
                            qt_t[:, (b * 4 + i) * L:(b * 4 + i + 1) * L],
                            start=(i == 0), stop=(i == 3))
                    dst = gcp[m0][:, b * L2:(b + 1) * L2] \
                        .rearrange("p (r l) -> p r l", r=2)
                    src = gp[:, :].unsqueeze(1).broadcast_to((mn, 2, L))
                    if ci == 0:
                        nc.vector.tensor_copy(dst, src)
                    else:
                        nc.scalar.activation(dst, src, CopyF, bias=0.0, scale=1.0)
                # partial Gsum as soon as pairs are ready (non-dup halves)
                if b == 1:
                    s01 = {}
                    for (m0, mn) in PCH:
                        t = sb.tile([mn, L], f32, tag=f"s01_{m0}", name=f"s01_{m0}")
                        nc.vector.tensor_add(t[:, :], gcp[m0][:, 0:L],
                                             gcp[m0][:, L2:L2 + L])
                        s01[m0] = t
                if b == 3:
                    for (m0, mn) in PCH:
                        t = sb.tile([mn, L], f32, tag=f"gsm_{m0}", name=f"gsm_{m0}")
                        nc.vector.tensor_add(t[:, :], gcp[m0][:, 2 * L2:2 * L2 + L],
                                             gcp[m0][:, 3 * L2:3 * L2 + L])
                        nc.vector.tensor_add(t[:, :], t[:, :], s01[m0])
                        gsm[m0] = t

            # ================= Gsum shear bounce -> part -> arin =================
            # write row p (duplicated 2x via stride-0 in-AP) at pitch 575 from
            # base 192; read back at pitch 576 -> gssh[p,t] = Gsum[p,(p+t)%L]
            engs = [nc.sync, nc.scalar]
            for ei, (m0, mn) in enumerate(PCH):
                engs[ei].dma_start(
                    out=AP(tensor=gs3, offset=192 + m0 * 575,
                           ap=[[575, mn], [L, 2], [1, L]]),
                    in_=gsm[m0][:, :].unsqueeze(1).broadcast_to((mn, 2, L)))
            gssh = {}
            for ei, (m0, mn) in enumerate(PCH):
                t = sb.tile([mn, L], f32, tag=f"gssh{m0}", name=f"gssh{m0}")
                engs[ei].dma_start(
                    out=t[:, :],
                    in_=AP(tensor=gs3, offset=192 + m0 * 576,
                           ap=[[576, mn], [1, L]]))
                gssh[m0] = t
            if USE_RDMA:
                # part replicated to all 128 partitions (payload for the
                # butterfly exchange; every row identical)
                partp = ps.tile([128, L], f32, tag="dsm", bufs=1)
                for ei, (m0, mn) in enumerate(PCH):
                    nc.tensor.matmul(partp[:, :], ones_t2[:mn, :], gssh[m0][:, :],
                                     start=(ei == 0), stop=(ei == 1))
                acc0 = sb.tile([128, L], f32, tag="acc0")
                arin_i = nc.vector.tensor_copy(acc0[:, :], partp[:, :])

                # ---- butterfly all-reduce over XOR partners 1, 2, 4 ----
                RDESTS = []
                for k, d in enumerate((1, 2, 4)):
                    r = [None] * 8
                    r[4 if d & 4 else 0] = (0, d)
                    RDESTS.append(r)
                # The rs[k] >= 2 waits are injected into the add instructions
                # AFTER scheduling (see below): Tile's single-core scheduling
                # sim cannot model remote semaphore bumps and would deadlock.
                accs = [acc0]
                for k in range(3):
                    nc.gpsimd.remote_dma_broadcast(
                        out_ap=inbox[k][:, :], in_ap=accs[k][:, :],
                        remote_sem=rs[k], local_sem=ls_sem, rdests=RDESTS[k])
                    nc.gpsimd.trigger_dma(count=None)
                    nxt = sb.tile([128, L], f32, tag=f"acc{k + 1}",
                                  name=f"acc{k + 1}")
                    gate = nc.gpsimd.nop(hint=f"bfly_wait{k}", nofuse=True)
                    rdma_posthoc.append((gate, rs[k]))
                    ai = nc.gpsimd.tensor_add(nxt[:, :], accs[k][:, :],
                                              inbox[k][:, :])
                    add_dep_helper(ai.ins, gate.ins, sync=False,
                                   reason="gate add on partner data arrival")
                    ci = nc.gpsimd.sem_clear(rs[k])
                    add_dep_helper(ci.ins, ai.ins, sync=False,
                                   reason="clear only after consumption")
                    accs.append(nxt)
                gsum_full = accs[3]
            else:
                partp = ps.tile([4, L], f32, tag="dsm", bufs=1)
                for ei, (m0, mn) in enumerate(PCH):
                    nc.tensor.matmul(partp[0:1, :], ones_t[:mn, :], gssh[m0][:, :],
                                     start=(ei == 0), stop=(ei == 1))
                part_sb = sb.tile([1, L], f32, tag="part")
                nc.vector.tensor_copy(part_sb[:, :], partp[0:1, :])
                arin_i = nc.sync.dma_start(out=arin[:, :], in_=part_sb[:, :])

                # ================= collective =================
                if USE_ALLGATHER:
                    cc = nc.gpsimd.collective_compute(
                        "AllGather", Alu.bypass,
                        replica_groups=[list(range(N_CORES))],
                        ins=[arin[:, :]], outs=[arout[:, :]])
                else:
                    cc = nc.gpsimd.collective_compute(
                        "AllReduce", Alu.add,
                        replica_groups=[list(range(N_CORES))],
                        ins=[arin[:, :]], outs=[arout[:, :]])

            # ============ CC bring-up window: W loads + g3a bounce + W phase ====
            # (the collective rendezvous takes ~20-30us after the trigger; all
            # of this runs in that window. The explicit deps on arin keep the
            # scheduler from hoisting these transfers ahead of the critical
            # gs3 -> part -> arin chain.)
            flight = []
            flight.append(nc.sync.dma_start(out=kf_t[:, :], in_=kf_d[:, :]))
            flight.append(nc.scalar.dma_start(out=vt_t[:, :], in_=vt_d[:, :]))
            flight.append(nc.sync.dma_start(out=qr0_t[:, :], in_=qr0_d[:, :]))
            flight.append(nc.scalar.dma_start(out=qr1_t[:, :], in_=qr1_d[:, :]))
            for ei, (m0, mn) in enumerate(PCH):
                flight.append(engs[ei].dma_start(
                    out=AP(tensor=g3a, offset=192 + m0 * 575,
                           ap=[[575, mn], [BSTR, PER], [1, L2]]),
                    in_=gcp[m0][:, :].rearrange("p (b l) -> p b l", b=PER)))
            gsh = {}
            for ei, (m0, mn) in enumerate(PCH):
                t = sb.tile([mn, PER * L], f32, tag=f"gsh{m0}", name=f"gsh{m0}")
                engs[ei].dma_start(
                    out=t[:, :].rearrange("p (b l) -> p b l", b=PER),
                    in_=AP(tensor=g3a, offset=192 + m0 * 576,
                           ap=[[576, mn], [BSTR, PER], [1, L]]))
                gsh[m0] = t
            for i in flight:
                add_dep_helper(i.ins, arin_i.ins, sync=True,
                               reason="keep pre-trigger DMA window clear")

            # W matmuls + dup copies
            wcp = {}
            for (m0, mn) in PCH:
                wcp[m0] = sb.tile([mn, PER * L2], bf, tag=f"wcp{m0}", name=f"wcp{m0}")
            for b in range(PER):
                for ci, (m0, mn) in enumerate(PCH):
                    wp = ps.tile([mn, L], f32, tag="mm", bufs=4)
                    for i in range(4):
                        nc.tensor.matmul(
                            wp[:, :],
                            kf_t[:, (b * 4 + i) * L + m0:(b * 4 + i) * L + m0 + mn],
                            vt_t[:, (b * 4 + i) * L:(b * 4 + i + 1) * L],
                            start=(i == 0), stop=(i == 3))
                    dst = wcp[m0][:, b * L2:(b + 1) * L2] \
                        .rearrange("p (r l) -> p r l", r=2)
                    src = wp[:, :].unsqueeze(1).broadcast_to((mn, 2, L))
                    if ci == 0:
                        nc.vector.tensor_copy(dst, src)
                    else:
                        nc.scalar.activation(dst, src, CopyF, bias=0.0, scale=1.0)
            for ei, (m0, mn) in enumerate(PCH):
                engs[ei].dma_start(
                    out=AP(tensor=w3, offset=192 + m0 * 575,
                           ap=[[575, mn], [BSTR, PER], [1, L2]]),
                    in_=wcp[m0][:, :].rearrange("p (b l) -> p b l", b=PER))
            wsh = {}
            for ei, (m0, mn) in enumerate(PCH):
                t = sb.tile([mn, PER * L], bf, tag=f"wsh{m0}", name=f"wsh{m0}")
                engs[ei].dma_start(
                    out=t[:, :].rearrange("p (b l) -> p b l", b=PER),
                    in_=AP(tensor=w3, offset=192 + m0 * 576,
                           ap=[[576, mn], [BSTR, PER], [1, L]]))
                wsh[m0] = t

            # per-batch D -> e4 (needed right after CC completes)
            d4p = ps.tile([4, L], f32, tag="dsm", bufs=1)
            nmm = 2 * PER
            k = 0
            for b in range(PER):
                for (m0, mn) in PCH:
                    nc.tensor.matmul(d4p[:, :], oh_t[:mn, 4 * b:4 * b + 4],
                                     gsh[m0][:, b * L:(b + 1) * L],
                                     start=(k == 0), stop=(k == nmm - 1))
                    k += 1
            e4 = sb.tile([PER, L], f32, tag="e4")
            nc.scalar.activation(e4[:, :], d4p[:, :], Exp, bias=0.0, scale=1.0 / HE)

            # ================= post-collective =================
            if USE_RDMA:
                gsrow = gsum_full
            else:
                gg = sb.tile([N_CORES if USE_ALLGATHER else 1, L], f32, tag="gg")
                nc.sync.dma_start(out=gg[:, :], in_=arout[:, :])
                gsum1 = sb.tile([1, L], f32, tag="gsum1")
                if USE_ALLGATHER:
                    # cross-rank sum via PE (partition reduction)
                    gsp = ps.tile([4, L], f32, tag="dsm", bufs=1)
                    nc.tensor.matmul(gsp[0:1, :], ones_t[:N_CORES, :], gg[:, :],
                                     start=True, stop=True)
                    nc.vector.tensor_copy(gsum1[:, :], gsp[0:1, :])
                else:
                    nc.vector.tensor_copy(gsum1[:, :], gg[:, :])
                gsrow = gsum1
            mx = sb.tile([1, 8], f32, tag="mx")
            nc.vector.max(out=mx[:, :], in_=gsrow[0:1, :])
            mk = sb.tile([1, L], f32, tag="mk")
            nc.vector.tensor_scalar(out=mk[:, :], in0=gsrow[0:1, :],
                                    scalar1=mx[:, 4:5], scalar2=None, op0=Alu.is_ge)
            mrep = ps.tile([4, L], f32, tag="dsm", bufs=1)
            nc.tensor.matmul(mrep[:, :], ones14[:, :], mk[:, :],
                             start=True, stop=True)
            # cm = masked unnormalized weights; the 1/Z normalization is folded
            # into the final output copies (out is linear in c)
            cm = sb.tile([PER, L], bf, tag="cm")
            nc.vector.tensor_tensor(out=cm[:, :], in0=e4[:, :], in1=mrep[:, :],
                                    op=Alu.mult)

            # circulant build: cm tripled to DRAM, read back shifted per partition
            nc.gpsimd.dma_start(
                out=AP(tensor=c3a, offset=0, ap=[[576, PER], [192, 3], [1, L]]),
                in_=cm[:, :].unsqueeze(1).broadcast_to((PER, 3, L)))
            # per-pair tiles: Tile deps are tile-granular, so separate tiles
            # let batch 0/1's T1 start without waiting for batch 2/3's reads
            h1p = {}
            for p0 in (0, 2):
                for ei, (m0, mn) in enumerate(PCH):
                    t = sb.tile([mn, 2 * L], bf, tag=f"h1_{p0}_{m0}",
                                name=f"h1_{p0}_{m0}")
                    engs[ei].dma_start(
                        out=t[:, :].rearrange("p (b l) -> p b l", b=2),
                        in_=AP(tensor=c3a, offset=1 + m0 + 576 * p0,
                               ap=[[1, mn], [576, 2], [1, L]]))
                    h1p[(p0, m0)] = t

            # normalization factors, replicated to 128 partitions (off critical)
            z = sb.tile([PER, 1], f32, tag="z")
            nc.vector.tensor_reduce(out=z[:, :], in_=cm[:, :], axis=Ax.X, op=Alu.add)
            zr = sb.tile([PER, 1], f32, tag="zr")
            nc.vector.reciprocal(zr[:, :], z[:, :])
            zrT = sb.tile([1, PER], f32, tag="zrT")
            nc.gpsimd.dma_start(out=zrT[:, :], in_=zr[:, :])
            ones1r = sb.tile([1, 128], f32, tag="ones1r")
            nc.vector.memset(ones1r[:, :], 1.0)
            zrp = ps.tile([128, PER], f32, tag="dsm", bufs=1)
            nc.tensor.matmul(zrp[:, :], ones1r[:, :], zrT[:, :],
                             start=True, stop=True)
            zrep = sb.tile([128, PER], f32, tag="zrep")
            nc.vector.tensor_copy(zrep[:, :], zrp[:, :])

            # T1 = HankelC^T @ shear(W); per-batch unshear bounce.
            # tdup/mrev are per-batch tiles so each batch's chain is
            # independent in Tile's dependency tracking.
            tdup = {}
            mrev = {}
            MSTR = BSTR
            for b in range(PER):
                p0 = (b // 2) * 2
                for ci, (m0, mn) in enumerate(PCH):
                    tp = ps.tile([mn, L], f32, tag="mm", bufs=4)
                    for i, (u0, un) in enumerate(PCH):
                        nc.tensor.matmul(
                            tp[:, :],
                            h1p[(p0, u0)][:, (b - p0) * L + m0:
                                          (b - p0) * L + m0 + mn],
                            wsh[u0][:, b * L:(b + 1) * L],
                            start=(i == 0), stop=(i == 1))
                    td = sb.tile([mn, L2], bf, tag=f"tdup{b}_{m0}",
                                 name=f"tdup{b}_{m0}")
                    tdup[(b, m0)] = td
                    dst = td[:, :].rearrange("p (r l) -> p r l", r=2)
                    src = tp[:, :].unsqueeze(1).broadcast_to((mn, 2, L))
                    if ci == 0:
                        nc.vector.tensor_copy(dst, src)
                    else:
                        nc.scalar.activation(dst, src, CopyF, bias=0.0, scale=1.0)
                for ei, (m0, mn) in enumerate(PCH):
                    engs[ei].dma_start(
                        out=AP(tensor=m3a, offset=b * MSTR + 191 + m0 * 575,
                               ap=[[575, mn], [1, L2]]),
                        in_=tdup[(b, m0)][:, :])
                for ei, (m0, mn) in enumerate(PCH):
                    t = sb.tile([mn, L], bf, tag=f"mrev{b}_{m0}",
                                name=f"mrev{b}_{m0}")
                    engs[ei].dma_start(
                        out=t[:, :],
                        in_=AP(tensor=m3a, offset=b * MSTR + 192 + m0 * 576,
                               ap=[[576, mn], [1, L]]))
                    mrev[(b, m0)] = t

            # keep the PE hot while waiting for the unshear round trips
            # (these read h1 so the scheduler cannot hoist them earlier; the
            # HAM clock gate needs ~3.4us of sustained activity for 2.4 GHz)
            for w in range(12):
                wp = ps.tile([128, 256], f32, tag="warm_p", bufs=1)
                nc.tensor.matmul(wp[:, :], h1p[(0, 0)][:, 0:128], warm_t[:, :],
                                 start=True, stop=True)

            # final: out[l,:] = (sum_g mrev[g,l] * qr[g,:]) / Z_b
            osum = {}
            for p0 in (0, 2):
                for (l0, ln) in PCH:
                    osum[(p0, l0)] = sb.tile([ln, 2 * HE], bf,
                                             tag=f"osum{p0}_{l0}",
                                             name=f"osum{p0}_{l0}")
            qrt = {0: qr0_t, 128: qr1_t}
            for b in range(PER):
                p0 = (b // 2) * 2
                for ci, (l0, ln) in enumerate(PCH):
                    op_ = ps.tile([ln, HE], f32, tag="op", bufs=2)
                    for i, (i0, in_n) in enumerate(PCH):
                        nc.tensor.matmul(op_[:, :],
                                         mrev[(b, i0)][:, l0: l0 + ln],
                                         qrt[i0][:, b * HE:(b + 1) * HE],
                                         start=(i == 0), stop=(i == 1))
                    dst = osum[(p0, l0)][:, (b - p0) * HE:(b - p0 + 1) * HE]
                    if ci == 0:
                        nc.vector.tensor_scalar(out=dst, in0=op_[:, :],
                                                scalar1=zrep[:ln, b:b + 1],
                                                scalar2=None, op0=Alu.mult)
                    else:
                        nc.scalar.activation(dst, op_[:, :], CopyF, bias=0.0,
                                             scale=zrep[:ln, b:b + 1])
                if b % 2 == 1:   # pair complete -> output write
                    for (l0, ln) in PCH:
                        nc.gpsimd.dma_start(
                            out=AP(tensor=out_d, offset=p0 * L * HE + l0 * HE,
                                   ap=[[HE, ln], [L * HE, 2], [1, HE]]),
                            in_=osum[(p0, l0)][:, :]
                                .rearrange("p (b h) -> p b h", b=2))

    # Inject cross-core semaphore waits post-scheduling (an extra wait only
    # delays an instruction, so Tile's semaphore protocol stays valid).
    for bi, sem in rdma_posthoc:
        bi.wait_op(sem, 2, "sem-ge")

    nc.finalize()
    return nc


def _get_nc():
    if "nc" not in _compiled:
        _compiled["nc"] = _build()
    return _compiled["nc"]


def kernel(queries, keys, values, adj, attn_mask):
    import ml_dtypes
    from concourse.bass_utils import run_bass_kernel_spmd

    bf16 = ml_dtypes.bfloat16
    queries = np.ascontiguousarray(np.asarray(queries, dtype=np.float32))
    keys = np.ascontiguousarray(np.asarray(keys, dtype=np.float32))
    values = np.ascontiguousarray(np.asarray(values, dtype=np.float32))

    def pack_proper(x):   # (PER,L,H,E) -> (128, PER*4*L): [p,(b*4+i)*L+s] = X[b,s,128i+p]
        t = x.reshape(PER, L, HE).transpose(0, 2, 1)
        t = t.reshape(PER, 4, 128, L).transpose(2, 0, 1, 3)
        return np.ascontiguousarray(t.reshape(128, PER * 4 * L)).astype(bf16)

    def pack_view(x):     # torch-style .view(HE, L) layout
        t = x.reshape(PER, HE, L)
        t = t.reshape(PER, 4, 128, L).transpose(2, 0, 1, 3)
        return np.ascontiguousarray(t.reshape(128, PER * 4 * L)).astype(bf16)

    def pack_qr(x):       # reversed (L,HE) per batch, split into row chunks
        t = x.reshape(PER, HE, L).transpose(0, 2, 1)[:, ::-1, :]
        a = np.ascontiguousarray(t[:, 0:128, :].transpose(1, 0, 2)
                                 .reshape(128, PER * HE)).astype(bf16)
        b = np.ascontiguousarray(t[:, 128:192, :].transpose(1, 0, 2)
                                 .reshape(64, PER * HE)).astype(bf16)
        return a, b

    nc = _get_nc()
    in_maps = []
    for c in range(N_CORES):
        sl = slice(c * PER, (c + 1) * PER)
        q, k, v = queries[sl], keys[sl], values[sl]
        qr0, qr1 = pack_qr(q)
        in_maps.append({
            "kt": pack_proper(k),
            "qt": pack_proper(q),
            "kf": pack_view(k),
            "vt": pack_proper(v),
            "qr0": qr0,
            "qr1": qr1,
        })

    res = run_bass_kernel_spmd(nc, in_maps, list(range(N_CORES)),
                               **_compiled.get("run_kwargs", {}))
    _compiled["last_result"] = res
    outs = [np.asarray(res.results[c]["out"]).astype(np.float32)
            .reshape(PER, L, H, E) for c in range(N_CORES)]
    return np.concatenate(outs, axis=0)


# revision 36
# speedup vs baseline: 1.0790x; 1.0790x over previous
"""Trainium2 Bass kernel for nn_AutoCorrelation_spa_tem.

Shards batch B=32 across 8 NeuronCores (4 batches/core, pure data parallel).

Algorithm (collapsed form of the reference):
  G_b   = keys[b](L,HE) @ queries[b](L,HE)^T            (192x192)
  D_raw[b,tau] = sum_s G_b[s,(s+tau)%L]                 (diag sums via shear)
  gsum  = AllGather_b(sum_b D_raw) + local sum -> top-5 mask via max8
  c_b   = mask * softmax(D_raw[b]/HE over selected)
  W_b   = keys[b].reshape(HE,L)^T @ values_proper(HE,L) (192x192)
  M_b   = sum_d c_b[d] * Shift_d(W_b)   [2D circular diagonal shift]
        = unshear(HankelC^T @ shear(W_b))   (all positive-stride DMAs)
  out[b] = (Qtilde_b @ M_b)^T  computed as Mrev^T @ qr  (qr host-row-reversed)

Perf structure vs the naive version:
  - all matmul operands in bf16 (tolerance is 2e-2; end-to-end err ~5e-3)
  - DMAs batched across the 4 local batches with 3-level access patterns
    (~30 dma_starts total; descriptor-generation on the issuing engine is
    ~0.6-1us per dma_start and was the old bottleneck)
  - collective input computed from Gsum = sum_b G_b (one small fp32 shear
    bounce) so the AllGather triggers as early as possible; everything else
    (per-batch D, W phase) runs during collective flight
  - dma_starts spread across sync(SP-HWDGE) / scalar(Act-HWDGE) / gpsimd
"""

import numpy as np

B, L, H, E = 32, 192, 8, 64
HE = H * E
N_CORES = 8
PER = B // N_CORES
L2 = 2 * L
BSTR = L * 576            # per-batch stride in shear scratch arrays

USE_ALLGATHER = True
USE_RDMA = False          # butterfly all-reduce via remote_dma: hits an opaque
                          # NRT INTERNAL error on this runtime; keep off
N_WARM = 8                # PE pstate warm-up matmuls

_compiled = {}


def _build():
    import concourse.bacc as bacc
    import concourse.mybir as mybir
    from concourse.bass_types import AP
    from concourse.tile import TileContext, add_dep_helper

    f32 = mybir.dt.float32
    bf = mybir.dt.bfloat16
    Exp = mybir.ActivationFunctionType.Exp
    CopyF = mybir.ActivationFunctionType.Copy
    Alu = mybir.AluOpType
    Ax = mybir.AxisListType

    nc = bacc.Bacc("TRN2", target_bir_lowering=False, debug=False,
                   num_devices=N_CORES, num_swdge_queues=2)

    # ---- dram I/O (host-packed layouts, see kernel()) ----
    kt_d = nc.dram_tensor("kt", [128, PER * 4 * L], bf, kind="ExternalInput")
    qt_d = nc.dram_tensor("qt", [128, PER * 4 * L], bf, kind="ExternalInput")
    kf_d = nc.dram_tensor("kf", [128, PER * 4 * L], bf, kind="ExternalInput")
    vt_d = nc.dram_tensor("vt", [128, PER * 4 * L], bf, kind="ExternalInput")
    qr0_d = nc.dram_tensor("qr0", [128, PER * HE], bf, kind="ExternalInput")
    qr1_d = nc.dram_tensor("qr1", [64, PER * HE], bf, kind="ExternalInput")
    out_d = nc.dram_tensor("out", [PER, L, HE], bf, kind="ExternalOutput")

    # ---- dram scratch ----
    gs3 = nc.dram_tensor("gs3", [L * 576 + 640], f32)          # Gsum shear bounce
    g3a = nc.dram_tensor("g3a", [PER * BSTR + 640], f32)       # per-batch G bounce
    w3 = nc.dram_tensor("w3", [PER * BSTR + 640], bf)          # W shear bounce
    m3a = nc.dram_tensor("m3a", [PER * BSTR + 640], bf)        # M unshear bounce
    c3a = nc.dram_tensor("c3a", [PER * 576 + 640], bf)         # tripled c for circulant
    arin = nc.dram_tensor("arin", [1, L], f32)
    if USE_ALLGATHER:
        arout = nc.dram_tensor("arout", [N_CORES, L], f32, addr_space="Shared")
    else:
        arout = nc.dram_tensor("arout", [1, L], f32, addr_space="Shared")

    PCH = [(0, 128), (128, 64)]
    rdma_posthoc = []

    with TileContext(nc) as tc:
        with tc.tile_pool(name="sb", bufs=1) as sb, \
             tc.tile_pool(name="ps", bufs=1, space="PSUM") as ps:

            # ================= constants (off critical path) =================
            warm_t = sb.tile([128, 256], bf, tag="warm_t")
            nc.vector.memset(warm_t[:, :], 0.125)
            ones_t = sb.tile([128, 1], f32, tag="ones")
            nc.vector.memset(ones_t[:, :], 1.0)
            oh_t = sb.tile([128, 16], f32, tag="oh")   # one-hot blocks: col 4b+b is 1
            nc.vector.memset(oh_t[:, :], 0.0)
            for b in range(PER):
                nc.vector.memset(oh_t[:, 4 * b + b: 4 * b + b + 1], 1.0)
            ones14 = sb.tile([1, 4], f32, tag="ones14")
            nc.vector.memset(ones14[:, :], 1.0)
            if USE_RDMA:
                ones_t2 = sb.tile([128, 128], f32, tag="ones_t2")
                nc.vector.memset(ones_t2[:, :], 1.0)
                # rs[k] is bumped only by the round-k partner; cleared right
                # after consumption (NOT at start: a partner leading by the
                # launch skew may bump before our preamble runs). Inboxes are
                # deliberately never written locally for the same reason.
                rs = [nc.alloc_semaphore(f"bfly_r{k}") for k in range(3)]
                ls_sem = nc.alloc_semaphore("bfly_l")
                inbox = []
                for k in range(3):
                    ib = sb.tile([128, L], f32, tag=f"inbox{k}", name=f"inbox{k}")
                    inbox.append(ib)

            # ================= input loads (G-phase operands only) =================
            HL = PER * 4 * L // 2
            kt_t = sb.tile([128, PER * 4 * L], bf, tag="kt_t")
            qt_t = sb.tile([128, PER * 4 * L], bf, tag="qt_t")
            kf_t = sb.tile([128, PER * 4 * L], bf, tag="kf_t")
            vt_t = sb.tile([128, PER * 4 * L], bf, tag="vt_t")
            qr0_t = sb.tile([128, PER * HE], bf, tag="qr0_t")
            qr1_t = sb.tile([64, PER * HE], bf, tag="qr1_t")
            nc.sync.dma_start(out=kt_t[:, 0:HL], in_=kt_d[:, 0:HL])
            nc.gpsimd.dma_start(out=qt_t[:, 0:HL], in_=qt_d[:, 0:HL])
            nc.sync.dma_start(out=kt_t[:, HL:], in_=kt_d[:, HL:])
            nc.gpsimd.dma_start(out=qt_t[:, HL:], in_=qt_d[:, HL:])

            # ================= PE warm-up =================
            for w in range(N_WARM):
                wp = ps.tile([128, 256], f32, tag="warm_p", bufs=1)
                nc.tensor.matmul(wp[:, :], warm_t[:, 0:128], warm_t[:, :],
                                 start=True, stop=True)

            # ================= G matmuls + dup copies + Gsum =================
            gcp = {}   # (chunk) -> tile (mn, PER*2L) f32, per-batch rows duplicated
            for ci, (m0, mn) in enumerate(PCH):
                gcp[m0] = sb.tile([mn, PER * L2], f32, tag=f"gcp{m0}", name=f"gcp{m0}")
            gsm = {}   # Gsum per chunk (mn, L) f32
            for b in range(PER):
                for ci, (m0, mn) in enumerate(PCH):
                    gp = ps.tile([mn, L], f32, tag="mm", bufs=4)
                    for i in range(4):
                        nc.tensor.matmul(
                            gp[:, :],
                            kt_t[:, (b * 4 + i) * L + m0:(b * 4 + i) * L + m0 + mn],
                            qt_t[:, (b * 4 + i) * L:(b * 4 + i + 1) * L],
                            start=(i == 0), stop=(i == 3))
                    dst = gcp[m0][:, b * L2:(b + 1) * L2] \
                        .rearrange("p (r l) -> p r l", r=2)
                    src = gp[:, :].unsqueeze(1).broadcast_to((mn, 2, L))
                    if ci == 0:
                        nc.vector.tensor_copy(dst, src)
                    else:
                        nc.scalar.activation(dst, src, CopyF, bias=0.0, scale=1.0)
                # partial Gsum as soon as pairs are ready (non-dup halves)
                if b == 1:
                    s01 = {}
                    for (m0, mn) in PCH:
                        t = sb.tile([mn, L], f32, tag=f"s01_{m0}", name=f"s01_{m0}")
                        nc.vector.tensor_add(t[:, :], gcp[m0][:, 0:L],
                                             gcp[m0][:, L2:L2 + L])
                        s01[m0] = t
                if b == 3:
                    for (m0, mn) in PCH:
                        t = sb.tile([mn, L], f32, tag=f"gsm_{m0}", name=f"gsm_{m0}")
                        nc.vector.tensor_add(t[:, :], gcp[m0][:, 2 * L2:2 * L2 + L],
                                             gcp[m0][:, 3 * L2:3 * L2 + L])
                        nc.vector.tensor_add(t[:, :], t[:, :], s01[m0])
                        gsm[m0] = t

            # ================= Gsum shear bounce -> part -> arin =================
            # write row p (duplicated 2x via stride-0 in-AP) at pitch 575 from
            # base 192; read back at pitch 576 -> gssh[p,t] = Gsum[p,(p+t)%L]
            engs = [nc.sync, nc.scalar]
            for ei, (m0, mn) in enumerate(PCH):
                engs[ei].dma_start(
                    out=AP(tensor=gs3, offset=192 + m0 * 575,
                           ap=[[575, mn], [L, 2], [1, L]]),
                    in_=gsm[m0][:, :].unsqueeze(1).broadcast_to((mn, 2, L)))
            gssh = {}
            for ei, (m0, mn) in enumerate(PCH):
                t = sb.tile([mn, L], f32, tag=f"gssh{m0}", name=f"gssh{m0}")
                engs[ei].dma_start(
                    out=t[:, :],
                    in_=AP(tensor=gs3, offset=192 + m0 * 576,
                           ap=[[576, mn], [1, L]]))
                gssh[m0] = t
            if USE_RDMA:
                # part replicated to all 128 partitions (payload for the
                # butterfly exchange; every row identical)
                partp = ps.tile([128, L], f32, tag="dsm", bufs=1)
                for ei, (m0, mn) in enumerate(PCH):
                    nc.tensor.matmul(partp[:, :], ones_t2[:mn, :], gssh[m0][:, :],
                                     start=(ei == 0), stop=(ei == 1))
                acc0 = sb.tile([128, L], f32, tag="acc0")
                arin_i = nc.vector.tensor_copy(acc0[:, :], partp[:, :])

                # ---- butterfly all-reduce over XOR partners 1, 2, 4 ----
                RDESTS = []
                for k, d in enumerate((1, 2, 4)):
                    r = [None] * 8
                    r[4 if d & 4 else 0] = (0, d)
                    RDESTS.append(r)
                # The rs[k] >= 2 waits are injected into the add instructions
                # AFTER scheduling (see below): Tile's single-core scheduling
                # sim cannot model remote semaphore bumps and would deadlock.
                accs = [acc0]
                for k in range(3):
                    nc.gpsimd.remote_dma_broadcast(
                        out_ap=inbox[k][:, :], in_ap=accs[k][:, :],
                        remote_sem=rs[k], local_sem=ls_sem, rdests=RDESTS[k])
                    nc.gpsimd.trigger_dma(count=None)
                    nxt = sb.tile([128, L], f32, tag=f"acc{k + 1}",
                                  name=f"acc{k + 1}")
                    gate = nc.gpsimd.nop(hint=f"bfly_wait{k}", nofuse=True)
                    rdma_posthoc.append((gate, rs[k]))
                    ai = nc.gpsimd.tensor_add(nxt[:, :], accs[k][:, :],
                                              inbox[k][:, :])
                    add_dep_helper(ai.ins, gate.ins, sync=False,
                                   reason="gate add on partner data arrival")
                    ci = nc.gpsimd.sem_clear(rs[k])
                    add_dep_helper(ci.ins, ai.ins, sync=False,
                                   reason="clear only after consumption")
                    accs.append(nxt)
                gsum_full = accs[3]
            else:
                partp = ps.tile([4, L], f32, tag="dsm", bufs=1)
                for ei, (m0, mn) in enumerate(PCH):
                    nc.tensor.matmul(partp[0:1, :], ones_t[:mn, :], gssh[m0][:, :],
                                     start=(ei == 0), stop=(ei == 1))
                part_sb = sb.tile([1, L], f32, tag="part")
                nc.vector.tensor_copy(part_sb[:, :], partp[0:1, :])
                arin_i = nc.sync.dma_start(out=arin[:, :], in_=part_sb[:, :])

                # ================= collective =================
                if USE_ALLGATHER:
                    cc = nc.gpsimd.collective_compute(
                        "AllGather", Alu.bypass,
                        replica_groups=[list(range(N_CORES))],
                        ins=[arin[:, :]], outs=[arout[:, :]])
                else:
                    cc = nc.gpsimd.collective_compute(
                        "AllReduce", Alu.add,
                        replica_groups=[list(range(N_CORES))],
                        ins=[arin[:, :]], outs=[arout[:, :]])

            # ============ CC bring-up window: W loads + g3a bounce + W phase ====
            # (the collective rendezvous takes ~20-30us after the trigger; all
            # of this runs in that window. The explicit deps on arin keep the
            # scheduler from hoisting these transfers ahead of the critical
            # gs3 -> part -> arin chain.)
            flight = []
            flight.append(nc.sync.dma_start(out=kf_t[:, :], in_=kf_d[:, :]))
            flight.append(nc.scalar.dma_start(out=vt_t[:, :], in_=vt_d[:, :]))
            flight.append(nc.sync.dma_start(out=qr0_t[:, :], in_=qr0_d[:, :]))
            flight.append(nc.scalar.dma_start(out=qr1_t[:, :], in_=qr1_d[:, :]))
            for ei, (m0, mn) in enumerate(PCH):
                flight.append(engs[ei].dma_start(
                    out=AP(tensor=g3a, offset=192 + m0 * 575,
                           ap=[[575, mn], [BSTR, PER], [1, L2]]),
                    in_=gcp[m0][:, :].rearrange("p (b l) -> p b l", b=PER)))
            gsh = {}
            for ei, (m0, mn) in enumerate(PCH):
                t = sb.tile([mn, PER * L], f32, tag=f"gsh{m0}", name=f"gsh{m0}")
                engs[ei].dma_start(
                    out=t[:, :].rearrange("p (b l) -> p b l", b=PER),
                    in_=AP(tensor=g3a, offset=192 + m0 * 576,
                           ap=[[576, mn], [BSTR, PER], [1, L]]))
                gsh[m0] = t
            for i in flight:
                add_dep_helper(i.ins, arin_i.ins, sync=True,
                               reason="keep pre-trigger DMA window clear")

            # W matmuls + dup copies
            wcp = {}
            for (m0, mn) in PCH:
                wcp[m0] = sb.tile([mn, PER * L2], bf, tag=f"wcp{m0}", name=f"wcp{m0}")
            for b in range(PER):
                for ci, (m0, mn) in enumerate(PCH):
                    wp = ps.tile([mn, L], f32, tag="mm", bufs=4)
                    for i in range(4):
                        nc.tensor.matmul(
                            wp[:, :],
                            kf_t[:, (b * 4 + i) * L + m0:(b * 4 + i) * L + m0 + mn],
                            vt_t[:, (b * 4 + i) * L:(b * 4 + i + 1) * L],
                            start=(i == 0), stop=(i == 3))
                    dst = wcp[m0][:, b * L2:(b + 1) * L2] \
                        .rearrange("p (r l) -> p r l", r=2)
                    src = wp[:, :].unsqueeze(1).broadcast_to((mn, 2, L))
                    if ci == 0:
                        nc.vector.tensor_copy(dst, src)
                    else:
                        nc.scalar.activation(dst, src, CopyF, bias=0.0, scale=1.0)
            for ei, (m0, mn) in enumerate(PCH):
                engs[ei].dma_start(
                    out=AP(tensor=w3, offset=192 + m0 * 575,
                           ap=[[575, mn], [BSTR, PER], [1, L2]]),
                    in_=wcp[m0][:, :].rearrange("p (b l) -> p b l", b=PER))
            wsh = {}
            for ei, (m0, mn) in enumerate(PCH):
                t = sb.tile([mn, PER * L], bf, tag=f"wsh{m0}", name=f"wsh{m0}")
                engs[ei].dma_start(
                    out=t[:, :].rearrange("p (b l) -> p b l", b=PER),
                    in_=AP(tensor=w3, offset=192 + m0 * 576,
                           ap=[[576, mn], [BSTR, PER], [1, L]]))
                wsh[m0] = t

            # per-batch D -> e4 (needed right after CC completes)
            d4p = ps.tile([4, L], f32, tag="dsm", bufs=1)
            nmm = 2 * PER
            k = 0
            for b in range(PER):
                for (m0, mn) in PCH:
                    nc.tensor.matmul(d4p[:, :], oh_t[:mn, 4 * b:4 * b + 4],
                                     gsh[m0][:, b * L:(b + 1) * L],
                                     start=(k == 0), stop=(k == nmm - 1))
                    k += 1
            e4 = sb.tile([PER, L], f32, tag="e4")
            nc.scalar.activation(e4[:, :], d4p[:, :], Exp, bias=0.0, scale=1.0 / HE)

            # ================= post-collective =================
            if USE_RDMA:
                gsrow = gsum_full
            else:
                gg = sb.tile([N_CORES if USE_ALLGATHER else 1, L], f32, tag="gg")
                nc.sync.dma_start(out=gg[:, :], in_=arout[:, :])
                gsum1 = sb.tile([1, L], f32, tag="gsum1")
                if USE_ALLGATHER:
                    # cross-rank sum via PE (partition reduction)
                    gsp = ps.tile([4, L], f32, tag="dsm", bufs=1)
                    nc.tensor.matmul(gsp[0:1, :], ones_t[:N_CORES, :], gg[:, :],
                                     start=True, stop=True)
                    nc.vector.tensor_copy(gsum1[:, :], gsp[0:1, :])
                else:
                    nc.vector.tensor_copy(gsum1[:, :], gg[:, :])
                gsrow = gsum1
            mx = sb.tile([1, 8], f32, tag="mx")
            nc.vector.max(out=mx[:, :], in_=gsrow[0:1, :])
            mk = sb.tile([1, L], f32, tag="mk")
            nc.vector.tensor_scalar(out=mk[:, :], in0=gsrow[0:1, :],
                                    scalar1=mx[:, 4:5], scalar2=None, op0=Alu.is_ge)
            mrep = ps.tile([4, L], f32, tag="dsm", bufs=1)
            nc.tensor.matmul(mrep[:, :], ones14[:, :], mk[:, :],
                             start=True, stop=True)
            # cm = masked unnormalized weights; the 1/Z normalization is folded
            # into the final output copies (out is linear in c)
            cm = sb.tile([PER, L], bf, tag="cm")
            nc.vector.tensor_tensor(out=cm[:, :], in0=e4[:, :], in1=mrep[:, :],
                                    op=Alu.mult)

            # circulant build: cm tripled to DRAM, read back shifted per partition
            nc.gpsimd.dma_start(
                out=AP(tensor=c3a, offset=0, ap=[[576, PER], [192, 3], [1, L]]),
                in_=cm[:, :].unsqueeze(1).broadcast_to((PER, 3, L)))
            # per-pair tiles: Tile deps are tile-granular, so separate tiles
            # let batch 0/1's T1 start without waiting for batch 2/3's reads
            h1p = {}
            for p0 in (0, 2):
                for ei, (m0, mn) in enumerate(PCH):
                    t = sb.tile([mn, 2 * L], bf, tag=f"h1_{p0}_{m0}",
                                name=f"h1_{p0}_{m0}")
                    engs[ei].dma_start(
                        out=t[:, :].rearrange("p (b l) -> p b l", b=2),
                        in_=AP(tensor=c3a, offset=1 + m0 + 576 * p0,
                               ap=[[1, mn], [576, 2], [1, L]]))
                    h1p[(p0, m0)] = t

            # normalization factors, replicated to 128 partitions (off critical)
            z = sb.tile([PER, 1], f32, tag="z")
            nc.vector.tensor_reduce(out=z[:, :], in_=cm[:, :], axis=Ax.X, op=Alu.add)
            zr = sb.tile([PER, 1], f32, tag="zr")
            nc.vector.reciprocal(zr[:, :], z[:, :])
            zrT = sb.tile([1, PER], f32, tag="zrT")
            nc.gpsimd.dma_start(out=zrT[:, :], in_=zr[:, :])
            ones1r = sb.tile([1, 128], f32, tag="ones1r")
            nc.vector.memset(ones1r[:, :], 1.0)
            zrp = ps.tile([128, PER], f32, tag="dsm", bufs=1)
            nc.tensor.matmul(zrp[:, :], ones1r[:, :], zrT[:, :],
                             start=True, stop=True)
            zrep = sb.tile([128, PER], f32, tag="zrep")
            nc.vector.tensor_copy(zrep[:, :], zrp[:, :])

            # T1 = HankelC^T @ shear(W); per-batch unshear bounce.
            # tdup/mrev are per-batch tiles so each batch's chain is
            # independent in Tile's dependency tracking.
            tdup = {}
            mrev = {}
            MSTR = BSTR
            for b in range(PER):
                p0 = (b // 2) * 2
                for ci, (m0, mn) in enumerate(PCH):
                    tp = ps.tile([mn, L], f32, tag="mm", bufs=4)
                    for i, (u0, un) in enumerate(PCH):
                        nc.tensor.matmul(
                            tp[:, :],
                            h1p[(p0, u0)][:, (b - p0) * L + m0:
                                          (b - p0) * L + m0 + mn],
                            wsh[u0][:, b * L:(b + 1) * L],
                            start=(i == 0), stop=(i == 1))
                    td = sb.tile([mn, L2], bf, tag=f"tdup{b}_{m0}",
                                 name=f"tdup{b}_{m0}")
                    tdup[(b, m0)] = td
                    dst = td[:, :].rearrange("p (r l) -> p r l", r=2)
                    src = tp[:, :].unsqueeze(1).broadcast_to((mn, 2, L))
                    if ci == 0:
                        nc.vector.tensor_copy(dst, src)
                    else:
                        nc.scalar.activation(dst, src, CopyF, bias=0.0, scale=1.0)
                for ei, (m0, mn) in enumerate(PCH):
                    engs[ei].dma_start(
                        out=AP(tensor=m3a, offset=b * MSTR + 191 + m0 * 575,
                               ap=[[575, mn], [1, L2]]),
                        in_=tdup[(b, m0)][:, :])
                for ei, (m0, mn) in enumerate(PCH):
                    t = sb.tile([mn, L], bf, tag=f"mrev{b}_{m0}",
                                name=f"mrev{b}_{m0}")
                    engs[ei].dma_start(
                        out=t[:, :],
                        in_=AP(tensor=m3a, offset=b * MSTR + 192 + m0 * 576,
                               ap=[[576, mn], [1, L]]))
                    mrev[(b, m0)] = t

            # keep the PE hot while waiting for the unshear round trips
            # (these read h1 so the scheduler cannot hoist them earlier; the
            # HAM clock gate needs ~3.4us of sustained activity for 2.4 GHz)
            for w in range(8):
                wp = ps.tile([128, 256], f32, tag="warm_p", bufs=1)
                nc.tensor.matmul(wp[:, :], h1p[(0, 0)][:, 0:128], warm_t[:, :],
                                 start=True, stop=True)

            # final: out[l,:] = (sum_g mrev[g,l] * qr[g,:]) / Z_b
            osum = {}
            for p0 in (0, 2):
                for (l0, ln) in PCH:
                    osum[(p0, l0)] = sb.tile([ln, 2 * HE], bf,
                                             tag=f"osum{p0}_{l0}",
                                             name=f"osum{p0}_{l0}")
            qrt = {0: qr0_t, 128: qr1_t}
            for b in range(PER):
                p0 = (b // 2) * 2
                for ci, (l0, ln) in enumerate(PCH):
                    op_ = ps.tile([ln, HE], f32, tag="op", bufs=2)
                    for i, (i0, in_n) in enumerate(PCH):
                        nc.tensor.matmul(op_[:, :],
                                         mrev[(b, i0)][:, l0: l0 + ln],
                                         qrt[i0][:, b * HE:(b + 1) * HE],
                                         start=(i == 0), stop=(i == 1))
                    dst = osum[(p0, l0)][:, (b - p0) * HE:(b - p0 + 1) * HE]
                    if ci == 0:
                        nc.vector.tensor_scalar(out=dst, in0=op_[:, :],
                                                scalar1=zrep[:ln, b:b + 1],
                                                scalar2=None, op0=Alu.mult)
                    else:
                        nc.scalar.activation(dst, op_[:, :], CopyF, bias=0.0,
                                             scale=zrep[:ln, b:b + 1])
                if b % 2 == 1:   # pair complete -> output write
                    for (l0, ln) in PCH:
                        nc.gpsimd.dma_start(
                            out=AP(tensor=out_d, offset=p0 * L * HE + l0 * HE,
                                   ap=[[HE, ln], [L * HE, 2], [1, HE]]),
                            in_=osum[(p0, l0)][:, :]
                                .rearrange("p (b h) -> p b h", b=2))

    # Inject cross-core semaphore waits post-scheduling (an extra wait only
    # delays an instruction, so Tile's semaphore protocol stays valid).
    for bi, sem in rdma_posthoc:
        bi.wait_op(sem, 2, "sem-ge")

    nc.finalize()
    return nc


def _get_nc():
    if "nc" not in _compiled:
        _compiled["nc"] = _build()
    return _compiled["nc"]


def kernel(queries, keys, values, adj, attn_mask):
    import ml_dtypes
    from concourse.bass_utils import run_bass_kernel_spmd

    bf16 = ml_dtypes.bfloat16
    queries = np.ascontiguousarray(np.asarray(queries, dtype=np.float32))
    keys = np.ascontiguousarray(np.asarray(keys, dtype=np.float32))
    values = np.ascontiguousarray(np.asarray(values, dtype=np.float32))

    def pack_proper(x):   # (PER,L,H,E) -> (128, PER*4*L): [p,(b*4+i)*L+s] = X[b,s,128i+p]
        t = x.reshape(PER, L, HE).transpose(0, 2, 1)
        t = t.reshape(PER, 4, 128, L).transpose(2, 0, 1, 3)
        return np.ascontiguousarray(t.reshape(128, PER * 4 * L)).astype(bf16)

    def pack_view(x):     # torch-style .view(HE, L) layout
        t = x.reshape(PER, HE, L)
        t = t.reshape(PER, 4, 128, L).transpose(2, 0, 1, 3)
        return np.ascontiguousarray(t.reshape(128, PER * 4 * L)).astype(bf16)

    def pack_qr(x):       # reversed (L,HE) per batch, split into row chunks
        t = x.reshape(PER, HE, L).transpose(0, 2, 1)[:, ::-1, :]
        a = np.ascontiguousarray(t[:, 0:128, :].transpose(1, 0, 2)
                                 .reshape(128, PER * HE)).astype(bf16)
        b = np.ascontiguousarray(t[:, 128:192, :].transpose(1, 0, 2)
                                 .reshape(64, PER * HE)).astype(bf16)
        return a, b

    nc = _get_nc()
    in_maps = []
    for c in range(N_CORES):
        sl = slice(c * PER, (c + 1) * PER)
        q, k, v = queries[sl], keys[sl], values[sl]
        qr0, qr1 = pack_qr(q)
        in_maps.append({
            "kt": pack_proper(k),
            "qt": pack_proper(q),
            "kf": pack_view(k),
            "vt": pack_proper(v),
            "qr0": qr0,
            "qr1": qr1,
        })

    res = run_bass_kernel_spmd(nc, in_maps, list(range(N_CORES)),
                               **_compiled.get("run_kwargs", {}))
    _compiled["last_result"] = res
    outs = [np.asarray(res.results[c]["out"]).astype(np.float32)
            .reshape(PER, L, H, E) for c in range(N_CORES)]
    return np.concatenate(outs, axis=0)
